# revision 9
# baseline (speedup 1.0000x reference)
"""Trainium2 Bass kernel for nn_CandidateFilterModel (segment_reduce).

Strategy (8 cores, S-column sharding for the heavy phases, pair sharding for
the tail):
  - Core k owns sequence-column slice s_k = [256k, 256k+256).
  - Host pre-gathers attention/sequence mention rows (index prep only), so the
    device phase-1 loads are plain sequential HWDGE DMAs, not SWDGE gathers.
  - Phase 1: entity aggregation via one-hot matmuls (zero et-blocks skipped
    using the sorted entity_ids); ent_att -> DRAM table, ent_emb^T via
    exp/matmul/log (logsumexp).
  - Phase 2: pair products. Indirect-gather ent_att rows of head/tail entity
    per pair tile (SWDGE), multiply + tree-reduce over 16 heads -> RAW,
    PE-transpose to RAW^T.
  - Phase 3: TWO AllToAll chunks (even pair-tiles fire at ~50% of phase 2) so
    most of the collective hides under the gathers.
  - Phase 4: pairs-local tail with full weights (bf16 from host on HWDGE):
    contexts, z_s/z_o, bilinear.
All matmuls bf16 (fp32 PSUM accumulate).
"""
import sys
import types
import numpy as np

S, H, HEADS = 2048, 1024, 16
E, NM, P = 256, 1024, 2048
PH = 1024
NC = 8
SL = S // NC          # 256 s-columns per core
PL = P // NC          # 256 pairs per core
NMT = NM // 128       # 8 mention tiles
NPT = P // 128        # 16 pair tiles
HS = HEADS * SL       # 4096

_CACHE = {}


def _ensure_axon_profile_hook():
    """bass_utils' trace path imports antenv.axon_hooks, absent in this image."""
    if 'antenv.axon_hooks' in sys.modules:
        return
    try:
        import antenv.axon_hooks  # noqa: F401
        return
    except ImportError:
        pass
    mod = types.ModuleType('antenv.axon_hooks')
    holder = [None]
    mod.set_axon_ntff_profile_hook = lambda h: holder.__setitem__(0, h)
    mod.get_axon_ntff_profile_hook = lambda: holder[0]
    sys.modules['antenv.axon_hooks'] = mod
    try:
        from trn_agent_boot.trn_boot import _ntff_profile_via_ctypes
        hook = _ntff_profile_via_ctypes('/opt/axon/libaxon_pjrt.so')
        if hook is not None:
            mod.set_axon_ntff_profile_hook(hook)
    except Exception:
        pass


def _build(mt_ets, debug=False):
    """mt_ets: per mention-tile tuple of entity-half chunks (0/1) it touches;
    derived from the sorted entity_ids, identical across cores."""
    import concourse.bass as bass
    import concourse.bacc as bacc
    import concourse.tile as tile
    from concourse import mybir
    from concourse.masks import make_identity

    F32 = mybir.dt.float32
    BF16 = mybir.dt.bfloat16
    I32 = mybir.dt.int32
    AF = mybir.ActivationFunctionType
    OP = mybir.AluOpType

    nc = bacc.Bacc(num_devices=NC)

    # ---------------- inputs ----------------
    attg = nc.declare_dram_parameter("attg", [128, NMT * HS], BF16, isOutput=False)
    vals = nc.declare_dram_parameter("vals", [128, NMT * H], BF16, isOutput=False)
    seqb = nc.declare_dram_parameter("seqb", [S, H], BF16, isOutput=False)
    p_off = nc.declare_dram_parameter("p_off", [128, 2 * NPT], I32, isOutput=False)
    ohe = nc.declare_dram_parameter("ohe", [NM, E], BF16, isOutput=False)
    ohm = nc.declare_dram_parameter("ohm", [NM, E], BF16, isOutput=False)
    has0r = nc.declare_dram_parameter("has0r", [1, E], F32, isOutput=False)
    ohh_k = nc.declare_dram_parameter("ohh_k", [E, PL], BF16, isOutput=False)
    oht_k = nc.declare_dram_parameter("oht_k", [E, PL], BF16, isOutput=False)
    w_head = nc.declare_dram_parameter("w_head", [H, PH], BF16, isOutput=False)
    w_tail = nc.declare_dram_parameter("w_tail", [H, PH], BF16, isOutput=False)
    w_ctx = nc.declare_dram_parameter("w_ctx", [H, PH], BF16, isOutput=False)
    w_bil = nc.declare_dram_parameter("w_bil", [PH, PH], BF16, isOutput=False)
    b_head = nc.declare_dram_parameter("b_head", [128, PH // 128], F32, isOutput=False)
    b_tail = nc.declare_dram_parameter("b_tail", [128, PH // 128], F32, isOutput=False)
    b_bil = nc.declare_dram_parameter("b_bil", [1, 1], F32, isOutput=False)
    out = nc.declare_dram_parameter("out", [1, PL], F32, isOutput=True)

    dbg = {}
    if debug:
        dbg["ent_embT"] = nc.declare_dram_parameter("d_ent_embT", [H, E], BF16, isOutput=True)
        dbg["entA"] = nc.declare_dram_parameter("d_entA", [E, HS], BF16, isOutput=True)
        dbg["raw"] = nc.declare_dram_parameter("d_raw", [128, NPT, SL], BF16, isOutput=True)
        dbg["ctxuT"] = nc.declare_dram_parameter("d_ctxuT", [H, PL], BF16, isOutput=True)
        dbg["zrow"] = nc.declare_dram_parameter("d_zrow", [1, PL], F32, isOutput=True)
        dbg["zsT"] = nc.declare_dram_parameter("d_zsT", [PH, PL], BF16, isOutput=True)

    # internal DRAM
    entA_dram = nc.dram_tensor("entA_dram", [E, HS], BF16)
    a2a_ev_in = nc.dram_tensor("a2a_ev_in", [NC, SL, 128], BF16)
    a2a_ev_out = nc.dram_tensor("a2a_ev_out", [NC, SL, 128], BF16)
    a2a_od_in = nc.dram_tensor("a2a_od_in", [NC, SL, 128], BF16)
    a2a_od_out = nc.dram_tensor("a2a_od_out", [NC, SL, 128], BF16)

    with tile.TileContext(nc) as tc:
        with tc.tile_pool(name="singles", bufs=1) as singles:
            entTe = singles.tile([128, H // 128, E], BF16)  # ent_emb^T [hcol-part, hc, e]
            RAW = singles.tile([128, NPT, SL], BF16)        # [p-row, pt, s]
            rawT = singles.tile([128, 2, NPT, 128], BF16)   # [s-part, sh, pt, p-row]
            paT = singles.tile([128, S // 128, PL], BF16)   # RAW^T for my pairs, all s
            ucb = singles.tile([128, H // 128, PL], BF16)   # contexts^T (unnormalized)
            ctxT = singles.tile([128, H // 128, PL], BF16)
            zsT = singles.tile([128, PH // 128, PL], BF16)
            zoT = singles.tile([128, PH // 128, PL], BF16)
            EWh = singles.tile([128, 2, PH], BF16)          # ent_emb @ W_head [e-part, et, PH]
            EWt = singles.tile([128, 2, PH], BF16)
            zrow = singles.tile([1, PL], F32)
            zrec = singles.tile([128, PL], BF16)
            lg_sb = singles.tile([1, PL], F32)

            # ---------------- phase 1: entity aggregation ----------------
            with tc.tile_pool(name="early", bufs=1) as early, \
                 tc.tile_pool(name="ps_a", bufs=1, space="PSUM") as ps_a, \
                 tc.tile_pool(name="wpoolA", bufs=1) as wpoolA:
                # one-hots first (small, needed by the first agg matmul), then
                # the big attg chunks; everything else rides the scalar queue
                ohm_t = early.tile([128, NMT, E], BF16)
                nc.sync.dma_start(out=ohm_t, in_=ohm.rearrange("(t p) e -> p t e", p=128))
                ohe_t = early.tile([128, NMT, E], BF16)
                nc.sync.dma_start(out=ohe_t, in_=ohe.rearrange("(t p) e -> p t e", p=128))
                attg_t = early.tile([128, NMT, HS], BF16)
                attg_v = attg.rearrange("p (t w) -> p t w", t=NMT)
                for hg in range(2):
                    nc.sync.dma_start(out=attg_t[:, :, hg * 2048:(hg + 1) * 2048],
                                      in_=attg_v[:, :, hg * 2048:(hg + 1) * 2048])
                vals_t = early.tile([128, NMT, H], BF16)
                nc.scalar.dma_start(out=vals_t, in_=vals.rearrange("p (t h) -> p t h", t=NMT))

                # ---------------- phase 0: small loads (scalar queue) -------
                p_off_t = singles.tile([128, 2 * NPT], I32)
                nc.scalar.dma_start(out=p_off_t, in_=p_off[:, :])
                ohh_t = singles.tile([128, 2, PL], BF16)
                nc.scalar.dma_start(out=ohh_t, in_=ohh_k.rearrange("(t p) q -> p t q", p=128))
                oht_t = singles.tile([128, 2, PL], BF16)
                nc.scalar.dma_start(out=oht_t, in_=oht_k.rearrange("(t p) q -> p t q", p=128))
                has0b = singles.tile([128, E], F32)
                nc.scalar.dma_start(out=has0b, in_=has0r[:, :].to_broadcast([128, E]))
                bhs_t = singles.tile([128, PH // 128], F32)
                nc.scalar.dma_start(out=bhs_t, in_=b_head[:, :])
                bts_t = singles.tile([128, PH // 128], F32)
                nc.scalar.dma_start(out=bts_t, in_=b_tail[:, :])
                bbil_t = singles.tile([1, 1], F32)
                nc.scalar.dma_start(out=bbil_t, in_=b_bil[:, :])
                whb = wpoolA.tile([128, H // 128, PH], BF16)
                nc.scalar.dma_start(out=whb, in_=w_head.rearrange("(t p) n -> p t n", p=128))
                wtb = wpoolA.tile([128, H // 128, PH], BF16)
                nc.scalar.dma_start(out=wtb, in_=w_tail.rearrange("(t p) n -> p t n", p=128))
                ident = singles.tile([128, 128], BF16)
                make_identity(nc, ident[:, :])
                warm = singles.tile([1, 8], F32)
                nc.vector.memset(warm[:, :], 0.0)
                nc.scalar.activation(out=warm[:, :], in_=warm[:, :], func=AF.Tanh)
                ones_col = singles.tile([128, 1], BF16)
                nc.vector.memset(ones_col[:, :], 1.0)
                ones_row = singles.tile([1, 128], BF16)
                nc.vector.memset(ones_row[:, :], 1.0)

                entA_sb = early.tile([128, 2, HS], BF16)  # [e-part, et, (h s)]
                ev = vals_t                               # exp applied in place

                # agg matmuls, skipping (mt, et) blocks that the sorted
                # entity_ids can never touch
                for hg in range(2):
                    for et in range(2):
                        mts = [mt for mt in range(NMT) if et in mt_ets[mt]]
                        if not mts:
                            nc.vector.memset(
                                entA_sb[:, et, hg * 2048:(hg + 1) * 2048], 0.0)
                        else:
                            pa = ps_a.tile([128, 8 * SL], F32, space="PSUM", tag="agg")
                            for i, mt in enumerate(mts):
                                for nch in range(4):  # 2048 = 4 x 512
                                    nc.tensor.matmul(
                                        pa[:, nch * 512:(nch + 1) * 512],
                                        ohm_t[:, mt, et * 128:(et + 1) * 128],
                                        attg_t[:, mt, hg * 2048 + nch * 512:
                                               hg * 2048 + (nch + 1) * 512],
                                        start=(i == 0), stop=(i == len(mts) - 1))
                            nc.vector.tensor_copy(
                                out=entA_sb[:, et, hg * 2048:(hg + 1) * 2048],
                                in_=pa[:, :])
                        nc.sync.dma_start(
                            out=entA_dram.rearrange("(t p) w -> p t w", p=128)[
                                :, et, hg * 2048:(hg + 1) * 2048],
                            in_=entA_sb[:, et, hg * 2048:(hg + 1) * 2048])
                if debug:
                    nc.sync.dma_start(
                        out=dbg["entA"].rearrange("(t p) w -> p t w", p=128), in_=entA_sb)

                # ---------------- phase 1b: logsumexp ----------------
                for mt in range(NMT):
                    nc.scalar.activation(out=ev[:, mt, :], in_=vals_t[:, mt, :],
                                         func=AF.Exp)
                with tc.tile_pool(name="ps_s", bufs=2, space="PSUM") as ps_s:
                    for hc in range(H // 128):
                        sp = ps_s.tile([128, E], F32, space="PSUM", tag="sums")
                        for mt in range(NMT):
                            nc.tensor.matmul(
                                sp[:, :], ev[:, mt, hc * 128:(hc + 1) * 128],
                                ohe_t[:, mt, :], start=(mt == 0), stop=(mt == NMT - 1))
                        nc.vector.tensor_tensor(out=sp[:, :], in0=sp[:, :],
                                                in1=has0b[:, :], op=OP.add)
                        nc.scalar.activation(out=entTe[:, hc, :], in_=sp[:, :], func=AF.Ln)
                if debug:
                    nc.sync.dma_start(
                        out=dbg["ent_embT"].rearrange("(t p) e -> p t e", p=128), in_=entTe)

                # ---------------- phase 4b: EW = ent_emb @ W (fills gather window) ----
                with tc.tile_pool(name="ps_e", bufs=2, space="PSUM") as ps_e:
                    for (wsb, dst) in ((whb, EWh), (wtb, EWt)):
                        for et in range(2):
                            ep = ps_e.tile([128, PH], F32, space="PSUM", tag="ew")
                            for kt in range(H // 128):
                                for nch in range(2):
                                    nc.tensor.matmul(
                                        ep[:, nch * 512:(nch + 1) * 512],
                                        entTe[:, kt, et * 128:(et + 1) * 128],
                                        wsb[:, kt, nch * 512:(nch + 1) * 512],
                                        start=(kt == 0), stop=(kt == H // 128 - 1))
                            nc.scalar.copy(out=dst[:, et, :], in_=ep[:, :])

            # ---------------- phase 2: pair products ----------------
            # pt order: evens then odds; the even-half AllToAll fires at ~50%.
            with tc.tile_pool(name="wpoolB", bufs=1) as wpoolB:
                wcb = wpoolB.tile([128, H // 128, PH], BF16)
                nc.scalar.dma_start(out=wcb, in_=w_ctx.rearrange("(t p) n -> p t n", p=128))
                seqx = wpoolB.tile([128, S // 128, H + 1], BF16)
                nc.scalar.dma_start(
                    out=seqx[:, :, 0:H], in_=seqb.rearrange("(t p) h -> p t h", p=128))
                nc.vector.memset(seqx[:, :, H:H + 1], 1.0)
                wbb = wpoolB.tile([128, PH // 128, PH], BF16)
                nc.scalar.dma_start(out=wbb, in_=w_bil.rearrange("(t p) n -> p t n", p=128))

                with tc.tile_pool(name="pg", bufs=3) as pg, \
                     tc.tile_pool(name="prod", bufs=1) as prod, \
                     tc.tile_pool(name="ps_t", bufs=4, space="PSUM") as ps_t:
                  for half, pts in (("ev", range(0, NPT, 2)), ("od", range(1, NPT, 2))):
                    for pt in pts:
                        th = pg.tile([128, HS], BF16, tag="th")
                        nc.gpsimd.indirect_dma_start(
                            out=th[:, :], out_offset=None, in_=entA_dram[:, :],
                            in_offset=bass.IndirectOffsetOnAxis(
                                ap=p_off_t[:, 2 * pt:2 * pt + 1], axis=0))
                        tt = pg.tile([128, HS], BF16, tag="tt")
                        nc.gpsimd.indirect_dma_start(
                            out=tt[:, :], out_offset=None, in_=entA_dram[:, :],
                            in_offset=bass.IndirectOffsetOnAxis(
                                ap=p_off_t[:, 2 * pt + 1:2 * pt + 2], axis=0))
                        pr = prod.tile([128, HS], BF16, tag="pr")
                        t1 = prod.tile([128, 8 * SL], BF16, tag="t1")
                        t2 = prod.tile([128, 4 * SL], BF16, tag="t2")
                        t3 = prod.tile([128, 2 * SL], BF16, tag="t3")
                        nc.vector.tensor_tensor(out=pr[:, :], in0=th[:, :], in1=tt[:, :],
                                                op=OP.mult)
                        nc.vector.tensor_tensor(out=t1[:, :], in0=pr[:, :8 * SL],
                                                in1=pr[:, 8 * SL:], op=OP.add)
                        nc.vector.tensor_tensor(out=t2[:, :], in0=t1[:, :4 * SL],
                                                in1=t1[:, 4 * SL:], op=OP.add)
                        nc.vector.tensor_tensor(out=t3[:, :], in0=t2[:, :2 * SL],
                                                in1=t2[:, 2 * SL:], op=OP.add)
                        nc.vector.tensor_tensor(out=RAW[:, pt, :], in0=t3[:, :SL],
                                                in1=t3[:, SL:], op=OP.add)
                        # transpose inline (PE is idle during products)
                        for sh in range(2):
                            tp = ps_t.tile([128, 128], BF16, space="PSUM", tag="tp")
                            nc.tensor.transpose(
                                out=tp[:, :], in_=RAW[:, pt, sh * 128:(sh + 1) * 128],
                                identity=ident[:, :])
                            nc.scalar.copy(out=rawT[:, sh, pt, :], in_=tp[:, :])
                        j = pt // 2
                        dst = a2a_ev_in if half == "ev" else a2a_od_in
                        nc.sync.dma_start(
                            out=dst[j].rearrange("(sh sp) q -> sp sh q", sh=2),
                            in_=rawT[:, :, pt, :])
                    # fire this half's AllToAll
                    if half == "ev":
                        nc.gpsimd.collective_compute(
                            "AllToAll", OP.bypass, replica_groups=[list(range(NC))],
                            ins=[a2a_ev_in[:, :, :]], outs=[a2a_ev_out[:, :, :]])
                        nc.sync.dma_start(
                            out=paT[:, :, 0:128],
                            in_=a2a_ev_out.rearrange("j (sh sp) q -> sp (j sh) q", sh=2))
                    else:
                        nc.gpsimd.collective_compute(
                            "AllToAll", OP.bypass, replica_groups=[list(range(NC))],
                            ins=[a2a_od_in[:, :, :]], outs=[a2a_od_out[:, :, :]])
                        nc.sync.dma_start(
                            out=paT[:, :, 128:256],
                            in_=a2a_od_out.rearrange("j (sh sp) q -> sp (j sh) q", sh=2))

                if debug:
                    nc.sync.dma_start(out=dbg["raw"].rearrange("p t s -> p (t s)"),
                                      in_=RAW.rearrange("p t s -> p (t s)"))

                # ---------------- phase 4: uc^T = [seq|1]^T @ pa ----------------
                with tc.tile_pool(name="ps_u", bufs=2, space="PSUM") as ps_u:
                    zp = ps_u.tile([1, PL], F32, space="PSUM", tag="zr")
                    for t in range(S // 128):
                        nc.tensor.matmul(
                            zp[:, :], seqx[:, t, H:H + 1], paT[:, t, :],
                            start=(t == 0), stop=(t == S // 128 - 1))
                    nc.vector.tensor_copy(out=zrow[:, :], in_=zp[:, :])
                    for mc in range(H // 128):
                        up = ps_u.tile([128, PL], F32, space="PSUM", tag="uc")
                        for t in range(S // 128):
                            nc.tensor.matmul(
                                up[:, :], seqx[:, t, mc * 128:(mc + 1) * 128],
                                paT[:, t, :], start=(t == 0), stop=(t == S // 128 - 1))
                        nc.vector.tensor_copy(out=ucb[:, mc, :], in_=up[:, :])
                if debug:
                    nc.sync.dma_start(
                        out=dbg["ctxuT"].rearrange("(t p) q -> p t q", p=128), in_=ucb)
                    nc.sync.dma_start(out=dbg["zrow"][:, :], in_=zrow)

                # recip(Z + 1e-6) -> broadcast to 128 partitions via K=1 matmul
                nc.vector.tensor_scalar_add(out=zrow[:, :], in0=zrow[:, :], scalar1=1e-6)
                nc.vector.reciprocal(out=zrow[:, :], in_=zrow[:, :])
                zrec_b = singles.tile([1, PL], BF16)
                nc.vector.tensor_copy(out=zrec_b, in_=zrow[:, :])
                with tc.tile_pool(name="ps_r", bufs=1, space="PSUM") as ps_r:
                    zrp = ps_r.tile([128, PL], F32, space="PSUM", tag="zrp")
                    nc.tensor.matmul(zrp[:, :], ones_row[:, :],
                                     zrec_b[:, :], start=True, stop=True)
                    nc.vector.tensor_copy(out=zrec, in_=zrp[:, :])
                for mc in range(H // 128):
                    nc.vector.tensor_tensor(out=ctxT[:, mc, :], in0=ucb[:, mc, :],
                                            in1=zrec[:, :], op=OP.mult)

                # ---------------- phase 5: z_s, z_o (ctx matmul shared) ----------------
                with tc.tile_pool(name="ps_z", bufs=2, space="PSUM") as ps_z, \
                     tc.tile_pool(name="zscr", bufs=2) as zscr:
                    for jt in range(PH // 128):
                        cps = ps_z.tile([128, PL], F32, space="PSUM", tag="cp")
                        for kt in range(H // 128):
                            nc.tensor.matmul(
                                cps[:, :], wcb[:, kt, jt * 128:(jt + 1) * 128],
                                ctxT[:, kt, :], start=(kt == 0), stop=(kt == H // 128 - 1))
                        cpsb = zscr.tile([128, PL], BF16, tag="cpsb")
                        nc.scalar.copy(out=cpsb[:, :], in_=cps[:, :])
                        for (ew, oh, bias, dst, tg) in ((EWh, ohh_t, bhs_t, zsT, "zs"),
                                                        (EWt, oht_t, bts_t, zoT, "zo")):
                            zps = ps_z.tile([128, PL], F32, space="PSUM", tag=tg)
                            for et in range(2):
                                nc.tensor.matmul(
                                    zps[:, :], ew[:, et, jt * 128:(jt + 1) * 128],
                                    oh[:, et, :], start=(et == 0), stop=(et == 1))
                            nc.vector.tensor_tensor(out=zps[:, :], in0=zps[:, :],
                                                    in1=cpsb[:, :], op=OP.add)
                            nc.scalar.activation(out=dst[:, jt, :], in_=zps[:, :],
                                                 func=AF.Tanh, bias=bias[:, jt:jt + 1])
                if debug:
                    nc.sync.dma_start(
                        out=dbg["zsT"].rearrange("(t p) q -> p t q", p=128), in_=zsT)
                # ---------------- phase 6: bilinear ----------------
                with tc.tile_pool(name="ps_b", bufs=3, space="PSUM") as ps_b, \
                     tc.tile_pool(name="bprod", bufs=2) as bprod:
                    lg = ps_b.tile([1, PL], F32, space="PSUM", tag="lg")
                    for jt in range(PH // 128):
                        ups = ps_b.tile([128, PL], F32, space="PSUM", tag="u")
                        for it in range(PH // 128):
                            nc.tensor.matmul(
                                ups[:, :], wbb[:, it, jt * 128:(jt + 1) * 128],
                                zsT[:, it, :], start=(it == 0), stop=(it == PH // 128 - 1))
                        pb = bprod.tile([128, PL], BF16, tag="pb")
                        nc.vector.tensor_tensor(out=pb[:, :], in0=ups[:, :],
                                                in1=zoT[:, jt, :], op=OP.mult)
                        nc.tensor.matmul(
                            lg[:, :], ones_col[:, :], pb[:, :],
                            start=(jt == 0), stop=(jt == PH // 128 - 1))
                    nc.vector.tensor_scalar_add(out=lg_sb[:, :], in0=lg[:, :],
                                                scalar1=bbil_t[:, 0:1])
                nc.sync.dma_start(out=out[:, :], in_=lg_sb)

    nc.finalize()
    return nc


def _get_nc(mt_ets, debug=False):
    key = ("nc", mt_ets, debug)
    if key not in _CACHE:
        _CACHE[key] = _build(mt_ets, debug)
    return _CACHE[key]


def _prep_in_maps(inputs):
    import ml_dtypes
    bf16 = ml_dtypes.bfloat16

    att = np.asarray(inputs["attention"], np.float32)          # [16, 2048, 2048]
    seq = np.ascontiguousarray(np.asarray(inputs["sequence_output"], np.float32))
    mention_idx = np.asarray(inputs["mention_idx"], np.int32)  # [1024]
    entity_ids = np.asarray(inputs["entity_ids"], np.int32)    # [1024]
    pair_h = np.asarray(inputs["pair_h"], np.int32)            # [2048]
    pair_t = np.asarray(inputs["pair_t"], np.int32)

    counts = np.bincount(entity_ids, minlength=E).astype(np.float32)
    inv_cnt = 1.0 / np.maximum(counts, 1.0)

    ohe = np.zeros((NM, E), np.float32)
    ohe[np.arange(NM), entity_ids] = 1.0
    ohm = np.zeros((NM, E), np.float32)
    ohm[np.arange(NM), entity_ids] = inv_cnt[entity_ids]
    has0r = (counts == 0).astype(np.float32)[None, :]

    # which entity-half chunks each mention tile touches (entity_ids sorted)
    eid_t = entity_ids.reshape(NMT, 128)
    mt_ets = tuple(
        tuple(sorted(set(int(e) // 128 for e in (eid_t[mt].min(), eid_t[mt].max()))
                     | set(range(int(eid_t[mt].min()) // 128,
                                 int(eid_t[mt].max()) // 128 + 1))))
        for mt in range(NMT))

    # pre-gathered mention rows: vals[p, mt*H:] = seq[mention_idx[mt*128+p]]
    vals_g = seq[mention_idx]                                   # [NM, H]
    vals_r = np.ascontiguousarray(
        vals_g.reshape(NMT, 128, H).transpose(1, 0, 2)).reshape(128, NMT * H).astype(bf16)

    # pre-gathered attention rows, s-sliced per core below
    attm = np.ascontiguousarray(
        att[:, mention_idx, :].transpose(1, 0, 2))              # [NM, HEADS, S]
    attm_bf = attm.astype(bf16)

    order = np.argsort(pair_h, kind="stable")
    sph = pair_h[order]
    spt = pair_t[order]
    p_off = np.zeros((128, 2 * NPT), np.int32)
    for pt in range(NPT):
        seg = slice(pt * 128, (pt + 1) * 128)
        p_off[:, 2 * pt] = sph[seg]
        p_off[:, 2 * pt + 1] = spt[seg]

    shared = {
        "vals": vals_r,
        "seqb": seq.astype(bf16),
        "p_off": p_off,
        "ohe": ohe.astype(bf16),
        "ohm": ohm.astype(bf16),
        "has0r": has0r,
        "w_head": np.asarray(inputs["W_head"], np.float32).astype(bf16),
        "w_tail": np.asarray(inputs["W_tail"], np.float32).astype(bf16),
        "w_ctx": np.asarray(inputs["W_ctx"], np.float32).astype(bf16),
        "w_bil": np.asarray(inputs["W_bil"], np.float32).astype(bf16),
        "b_head": np.asarray(inputs["b_head"], np.float32).reshape(PH // 128, 128).T.copy(),
        "b_tail": np.asarray(inputs["b_tail"], np.float32).reshape(PH // 128, 128).T.copy(),
        "b_bil": np.asarray(inputs["b_bil"], np.float32).reshape(1, 1),
    }

    in_maps = []
    for k in range(NC):
        sk = k * SL
        # attg[p, mt, (h s)] = attm[mt*128+p, :, sk:sk+SL]
        attg_k = np.ascontiguousarray(
            attm_bf[:, :, sk:sk + SL].reshape(NMT, 128, HS).transpose(1, 0, 2)
        ).reshape(128, NMT * HS)
        ohh_kk = np.zeros((E, PL), np.float32)
        ohh_kk[sph[k * PL:(k + 1) * PL], np.arange(PL)] = 1.0
        oht_kk = np.zeros((E, PL), np.float32)
        oht_kk[spt[k * PL:(k + 1) * PL], np.arange(PL)] = 1.0
        m = dict(shared)
        m["attg"] = attg_k
        m["ohh_k"] = ohh_kk.astype(bf16)
        m["oht_k"] = oht_kk.astype(bf16)
        in_maps.append(m)
    return in_maps, mt_ets


def _run(inputs, trace=False, debug=False):
    _ensure_axon_profile_hook()
    from concourse.bass_utils import run_bass_kernel_spmd
    in_maps, mt_ets = _prep_in_maps(inputs)
    nc = _get_nc(mt_ets, debug)
    res = run_bass_kernel_spmd(nc, in_maps, list(range(NC)), trace=trace)
    sorted_logits = np.concatenate([np.asarray(res.results[k]["out"][0], np.float32)
                                    for k in range(NC)])
    order = np.argsort(np.asarray(inputs["pair_h"], np.int32), kind="stable")
    logits = np.empty(P, np.float32)
    logits[order] = sorted_logits
    return logits, res


def kernel(**inputs) -> np.ndarray:
    logits, _ = _run(inputs, trace=False)
    return logits


def kernel_traced(**inputs):
    logits, res = _run(inputs, trace=True)
    return logits, res


def kernel_debug(**inputs):
    logits, res = _run(inputs, trace=False, debug=True)
    return logits, res


# revision 18
# speedup vs baseline: 1.1887x; 1.1887x over previous
"""Trainium2 Bass kernel for nn_CandidateFilterModel (segment_reduce).

Strategy (8 cores, S-column sharding for the heavy phases, pair sharding for
the tail):
  - Core k owns sequence-column slice s_k = [256k, 256k+256).
  - Host pre-gathers attention/sequence mention rows (index prep only), so the
    device phase-1 loads are plain sequential HWDGE DMAs, not SWDGE gathers.
  - Phase 1: entity aggregation via one-hot matmuls (zero et-blocks skipped
    using the sorted entity_ids); ent_att -> DRAM table, ent_emb^T via
    exp/matmul/log (logsumexp).
  - Phase 2: pair products. Indirect-gather ent_att rows of head/tail entity
    per pair tile (SWDGE), multiply + tree-reduce over 16 heads -> RAW,
    PE-transpose to RAW^T.
  - Phase 3: TWO AllToAll chunks (even pair-tiles fire at ~50% of phase 2) so
    most of the collective hides under the gathers.
  - Phase 4: pairs-local tail with full weights (bf16 from host on HWDGE):
    contexts, z_s/z_o, bilinear.
All matmuls bf16 (fp32 PSUM accumulate).
"""
import sys
import types
import numpy as np

S, H, HEADS = 2048, 1024, 16
E, NM, P = 256, 1024, 2048
PH = 1024
NC = 8
SL = S // NC          # 256 s-columns per core
PL = P // NC          # 256 pairs per core
NMT = NM // 128       # 8 mention tiles
NPT = P // 128        # 16 pair tiles
HS = HEADS * SL       # 4096

_CACHE = {}


def _ensure_axon_profile_hook():
    """bass_utils' trace path imports antenv.axon_hooks, absent in this image."""
    if 'antenv.axon_hooks' in sys.modules:
        return
    try:
        import antenv.axon_hooks  # noqa: F401
        return
    except ImportError:
        pass
    mod = types.ModuleType('antenv.axon_hooks')
    holder = [None]
    mod.set_axon_ntff_profile_hook = lambda h: holder.__setitem__(0, h)
    mod.get_axon_ntff_profile_hook = lambda: holder[0]
    sys.modules['antenv.axon_hooks'] = mod
    try:
        from trn_agent_boot.trn_boot import _ntff_profile_via_ctypes
        hook = _ntff_profile_via_ctypes('/opt/axon/libaxon_pjrt.so')
        if hook is not None:
            mod.set_axon_ntff_profile_hook(hook)
    except Exception:
        pass


def _build(mt_ets, debug=False):
    """mt_ets: per mention-tile tuple of entity-half chunks (0/1) it touches;
    derived from the sorted entity_ids, identical across cores."""
    import concourse.bass as bass
    import concourse.bacc as bacc
    import concourse.tile as tile
    from concourse import mybir
    from concourse.masks import make_identity

    F32 = mybir.dt.float32
    BF16 = mybir.dt.bfloat16
    FP8 = mybir.dt.float8e4
    I32 = mybir.dt.int32
    AF = mybir.ActivationFunctionType
    OP = mybir.AluOpType
    DR = mybir.MatmulPerfMode.DoubleRow

    nc = bacc.Bacc(num_devices=NC)

    # ---------------- inputs ----------------
    attg = nc.declare_dram_parameter("attg", [128, NMT * HS], FP8, isOutput=False)
    vals = nc.declare_dram_parameter("vals", [128, NMT * H], BF16, isOutput=False)
    seqb = nc.declare_dram_parameter("seqb", [S, H], BF16, isOutput=False)
    p_off = nc.declare_dram_parameter("p_off", [128, 2 * NPT], I32, isOutput=False)
    ohe = nc.declare_dram_parameter("ohe", [NM, E], BF16, isOutput=False)
    ohm = nc.declare_dram_parameter("ohm", [NM, E], FP8, isOutput=False)
    invc = nc.declare_dram_parameter("invc", [128, 2], F32, isOutput=False)
    has0r = nc.declare_dram_parameter("has0r", [1, E], F32, isOutput=False)
    ohh_k = nc.declare_dram_parameter("ohh_k", [E, PL], BF16, isOutput=False)
    oht_k = nc.declare_dram_parameter("oht_k", [E, PL], BF16, isOutput=False)
    w_head = nc.declare_dram_parameter("w_head", [H, PH], BF16, isOutput=False)
    w_tail = nc.declare_dram_parameter("w_tail", [H, PH], BF16, isOutput=False)
    w_ctx = nc.declare_dram_parameter("w_ctx", [H, PH], BF16, isOutput=False)
    w_bil = nc.declare_dram_parameter("w_bil", [PH, PH], BF16, isOutput=False)
    b_head = nc.declare_dram_parameter("b_head", [128, PH // 128], F32, isOutput=False)
    b_tail = nc.declare_dram_parameter("b_tail", [128, PH // 128], F32, isOutput=False)
    b_bil = nc.declare_dram_parameter("b_bil", [1, 1], F32, isOutput=False)
    out = nc.declare_dram_parameter("out", [1, PL], F32, isOutput=True)

    dbg = {}
    if debug:
        dbg["ent_embT"] = nc.declare_dram_parameter("d_ent_embT", [H, E], BF16, isOutput=True)
        dbg["entA"] = nc.declare_dram_parameter("d_entA", [E, HS], FP8, isOutput=True)
        dbg["raw"] = nc.declare_dram_parameter("d_raw", [128, NPT, SL], BF16, isOutput=True)
        dbg["ctxuT"] = nc.declare_dram_parameter("d_ctxuT", [H, PL], BF16, isOutput=True)
        dbg["zrow"] = nc.declare_dram_parameter("d_zrow", [1, PL], F32, isOutput=True)
        dbg["zsT"] = nc.declare_dram_parameter("d_zsT", [PH, PL], BF16, isOutput=True)

    # internal DRAM
    entA_dram = nc.dram_tensor("entA_dram", [E, HS], FP8)
    a2a_ev_in = nc.dram_tensor("a2a_ev_in", [NC, SL, 128], BF16)
    a2a_ev_out = nc.dram_tensor("a2a_ev_out", [NC, SL, 128], BF16)
    a2a_od_in = nc.dram_tensor("a2a_od_in", [NC, SL, 128], BF16)
    a2a_od_out = nc.dram_tensor("a2a_od_out", [NC, SL, 128], BF16)

    with tile.TileContext(nc) as tc:
        with tc.tile_pool(name="singles", bufs=1) as singles:
            entTe = singles.tile([128, H // 128, E], BF16)  # ent_emb^T [hcol-part, hc, e]
            RAW = singles.tile([128, NPT, SL], BF16)        # [p-row, pt, s]
            rawT = singles.tile([128, 2, NPT, 128], BF16)   # [s-part, sh, pt, p-row]
            paT = singles.tile([128, S // 128, PL], BF16)   # RAW^T for my pairs, all s
            ucb = singles.tile([128, H // 128, PL], BF16)   # contexts^T (unnormalized)
            ctxT = singles.tile([128, H // 128, PL], BF16)
            zsT = singles.tile([128, PH // 128, PL], BF16)
            zoT = singles.tile([128, PH // 128, PL], BF16)
            EWh = singles.tile([128, 2, PH], BF16)          # ent_emb @ W_head [e-part, et, PH]
            EWt = singles.tile([128, 2, PH], BF16)
            zrow = singles.tile([1, PL], F32)
            zrec = singles.tile([128, PL], BF16)
            lg_sb = singles.tile([1, PL], F32)

            # ---------------- phase 1: entity aggregation ----------------
            with tc.tile_pool(name="early", bufs=1) as early, \
                 tc.tile_pool(name="ps_a", bufs=1, space="PSUM") as ps_a, \
                 tc.tile_pool(name="wpoolA", bufs=1) as wpoolA:
                # one-hots first (small, needed by the first agg matmul), then
                # the big attg chunks; everything else rides the scalar queue
                ohm_t = early.tile([128, NMT, E], FP8)
                nc.sync.dma_start(out=ohm_t, in_=ohm.rearrange("(t p) e -> p t e", p=128))
                ohe_t = early.tile([128, NMT, E], BF16)
                nc.sync.dma_start(out=ohe_t, in_=ohe.rearrange("(t p) e -> p t e", p=128))
                invc_t = singles.tile([128, 2], F32)
                nc.sync.dma_start(out=invc_t, in_=invc[:, :])
                attg_t = early.tile([128, NMT, HS], FP8)
                attg_v = attg.rearrange("p (t w) -> p t w", t=NMT)
                for hg in range(2):
                    nc.sync.dma_start(out=attg_t[:, :, hg * 2048:(hg + 1) * 2048],
                                      in_=attg_v[:, :, hg * 2048:(hg + 1) * 2048])
                vals_t = early.tile([128, NMT, H], BF16)
                nc.scalar.dma_start(out=vals_t, in_=vals.rearrange("p (t h) -> p t h", t=NMT))

                # ---------------- phase 0: small loads (scalar queue) -------
                p_off_t = singles.tile([128, 2 * NPT], I32)
                nc.scalar.dma_start(out=p_off_t, in_=p_off[:, :])
                ohh_t = singles.tile([128, 2, PL], BF16)
                nc.scalar.dma_start(out=ohh_t, in_=ohh_k.rearrange("(t p) q -> p t q", p=128))
                oht_t = singles.tile([128, 2, PL], BF16)
                nc.scalar.dma_start(out=oht_t, in_=oht_k.rearrange("(t p) q -> p t q", p=128))
                has0b = singles.tile([128, E], F32)
                nc.scalar.dma_start(out=has0b, in_=has0r[:, :].to_broadcast([128, E]))
                bhs_t = singles.tile([128, PH // 128], F32)
                nc.scalar.dma_start(out=bhs_t, in_=b_head[:, :])
                bts_t = singles.tile([128, PH // 128], F32)
                nc.scalar.dma_start(out=bts_t, in_=b_tail[:, :])
                bbil_t = singles.tile([1, 1], F32)
                nc.scalar.dma_start(out=bbil_t, in_=b_bil[:, :])
                whb = wpoolA.tile([128, H // 128, PH], BF16)
                nc.scalar.dma_start(out=whb, in_=w_head.rearrange("(t p) n -> p t n", p=128))
                wtb = wpoolA.tile([128, H // 128, PH], BF16)
                nc.scalar.dma_start(out=wtb, in_=w_tail.rearrange("(t p) n -> p t n", p=128))
                ident = singles.tile([128, 128], BF16)
                make_identity(nc, ident[:, :])
                warm = singles.tile([1, 8], F32)
                nc.vector.memset(warm[:, :], 0.0)
                nc.scalar.activation(out=warm[:, :], in_=warm[:, :], func=AF.Tanh)
                ones_col = singles.tile([128, 1], BF16)
                nc.vector.memset(ones_col[:, :], 1.0)
                ones_row = singles.tile([1, 128], BF16)
                nc.vector.memset(ones_row[:, :], 1.0)

                entA_sb = early.tile([128, 2, HS], FP8)   # [e-part, et, (h s)]
                ev = vals_t                               # exp applied in place

                # agg matmuls (fp8 DoubleRow over mention-tile pairs), skipping
                # (mt, et) blocks that the sorted entity_ids can never touch
                for hg in range(2):
                    for et in range(2):
                        mts = [mt for mt in range(NMT) if et in mt_ets[mt]]
                        groups = []
                        i = 0
                        while i < len(mts):
                            if i + 1 < len(mts) and mts[i + 1] == mts[i] + 1:
                                groups.append((mts[i], 2))
                                i += 2
                            else:
                                groups.append((mts[i], 1))
                                i += 1
                        if not groups:
                            nc.vector.memset(
                                entA_sb[:, et, hg * 2048:(hg + 1) * 2048], 0.0)
                        else:
                            pa = ps_a.tile([128, 8 * SL], F32, space="PSUM", tag="agg")
                            for i, (mt, w) in enumerate(groups):
                                for nch in range(4):  # 2048 = 4 x 512
                                    cs = hg * 2048 + nch * 512
                                    if w == 2:
                                        nc.tensor.matmul(
                                            pa[:, nch * 512:(nch + 1) * 512],
                                            ohm_t[:, mt:mt + 2, et * 128:(et + 1) * 128],
                                            attg_t[:, mt:mt + 2, cs:cs + 512],
                                            start=(i == 0), stop=(i == len(groups) - 1),
                                            perf_mode=DR)
                                    else:
                                        nc.tensor.matmul(
                                            pa[:, nch * 512:(nch + 1) * 512],
                                            ohm_t[:, mt, et * 128:(et + 1) * 128],
                                            attg_t[:, mt, cs:cs + 512],
                                            start=(i == 0), stop=(i == len(groups) - 1))
                            nc.vector.tensor_scalar_mul(
                                out=entA_sb[:, et, hg * 2048:(hg + 1) * 2048],
                                in0=pa[:, :], scalar1=invc_t[:, et:et + 1])
                        nc.sync.dma_start(
                            out=entA_dram.rearrange("(t p) w -> p t w", p=128)[
                                :, et, hg * 2048:(hg + 1) * 2048],
                            in_=entA_sb[:, et, hg * 2048:(hg + 1) * 2048])
                if debug:
                    nc.sync.dma_start(
                        out=dbg["entA"].rearrange("(t p) w -> p t w", p=128), in_=entA_sb)

                # ---------------- phase 1b: logsumexp ----------------
                for mt in range(NMT):
                    nc.scalar.activation(out=ev[:, mt, :], in_=vals_t[:, mt, :],
                                         func=AF.Exp)
                with tc.tile_pool(name="ps_s", bufs=2, space="PSUM") as ps_s:
                    for hc in range(H // 128):
                        sp = ps_s.tile([128, E], F32, space="PSUM", tag="sums")
                        for mt in range(NMT):
                            nc.tensor.matmul(
                                sp[:, :], ev[:, mt, hc * 128:(hc + 1) * 128],
                                ohe_t[:, mt, :], start=(mt == 0), stop=(mt == NMT - 1))
                        nc.vector.tensor_tensor(out=sp[:, :], in0=sp[:, :],
                                                in1=has0b[:, :], op=OP.add)
                        nc.scalar.activation(out=entTe[:, hc, :], in_=sp[:, :], func=AF.Ln)
                if debug:
                    nc.sync.dma_start(
                        out=dbg["ent_embT"].rearrange("(t p) e -> p t e", p=128), in_=entTe)

                # ---------------- phase 4b: EW = ent_emb @ W (fills gather window) ----
                with tc.tile_pool(name="ps_e", bufs=2, space="PSUM") as ps_e:
                    for (wsb, dst) in ((whb, EWh), (wtb, EWt)):
                        for et in range(2):
                            ep = ps_e.tile([128, PH], F32, space="PSUM", tag="ew")
                            for kt in range(H // 128):
                                for nch in range(2):
                                    nc.tensor.matmul(
                                        ep[:, nch * 512:(nch + 1) * 512],
                                        entTe[:, kt, et * 128:(et + 1) * 128],
                                        wsb[:, kt, nch * 512:(nch + 1) * 512],
                                        start=(kt == 0), stop=(kt == H // 128 - 1))
                            nc.scalar.copy(out=dst[:, et, :], in_=ep[:, :])

            # ---------------- phase 2: pair products ----------------
            # pt order: evens then odds; the even-half AllToAll fires at ~50%.
            with tc.tile_pool(name="wpoolB", bufs=1) as wpoolB:
                wcb = wpoolB.tile([128, H // 128, PH], BF16)
                nc.scalar.dma_start(out=wcb, in_=w_ctx.rearrange("(t p) n -> p t n", p=128))
                seqx = wpoolB.tile([128, S // 128, H + 1], BF16)
                nc.scalar.dma_start(
                    out=seqx[:, :, 0:H], in_=seqb.rearrange("(t p) h -> p t h", p=128))
                nc.vector.memset(seqx[:, :, H:H + 1], 1.0)
                wbb = wpoolB.tile([128, PH // 128, PH], BF16)
                nc.scalar.dma_start(out=wbb, in_=w_bil.rearrange("(t p) n -> p t n", p=128))

                with tc.tile_pool(name="pg", bufs=3) as pg, \
                     tc.tile_pool(name="prod", bufs=1) as prod, \
                     tc.tile_pool(name="ps_t", bufs=4, space="PSUM") as ps_t:
                  for half, pts in (("ev", range(0, NPT, 2)), ("od", range(1, NPT, 2))):
                    for pt in pts:
                        th = pg.tile([128, HS], FP8, tag="th")
                        nc.gpsimd.indirect_dma_start(
                            out=th[:, :], out_offset=None, in_=entA_dram[:, :],
                            in_offset=bass.IndirectOffsetOnAxis(
                                ap=p_off_t[:, 2 * pt:2 * pt + 1], axis=0))
                        tt = pg.tile([128, HS], FP8, tag="tt")
                        nc.gpsimd.indirect_dma_start(
                            out=tt[:, :], out_offset=None, in_=entA_dram[:, :],
                            in_offset=bass.IndirectOffsetOnAxis(
                                ap=p_off_t[:, 2 * pt + 1:2 * pt + 2], axis=0))
                        pr = prod.tile([128, HS], BF16, tag="pr")
                        t1 = prod.tile([128, 8 * SL], BF16, tag="t1")
                        t2 = prod.tile([128, 4 * SL], BF16, tag="t2")
                        t3 = prod.tile([128, 2 * SL], BF16, tag="t3")
                        nc.vector.tensor_tensor(out=pr[:, :], in0=th[:, :], in1=tt[:, :],
                                                op=OP.mult)
                        nc.vector.tensor_tensor(out=t1[:, :], in0=pr[:, :8 * SL],
                                                in1=pr[:, 8 * SL:], op=OP.add)
                        nc.vector.tensor_tensor(out=t2[:, :], in0=t1[:, :4 * SL],
                                                in1=t1[:, 4 * SL:], op=OP.add)
                        nc.vector.tensor_tensor(out=t3[:, :], in0=t2[:, :2 * SL],
                                                in1=t2[:, 2 * SL:], op=OP.add)
                        nc.vector.tensor_tensor(out=RAW[:, pt, :], in0=t3[:, :SL],
                                                in1=t3[:, SL:], op=OP.add)
                        # transpose inline (PE is idle during products)
                        for sh in range(2):
                            tp = ps_t.tile([128, 128], BF16, space="PSUM", tag="tp")
                            nc.tensor.transpose(
                                out=tp[:, :], in_=RAW[:, pt, sh * 128:(sh + 1) * 128],
                                identity=ident[:, :])
                            nc.scalar.copy(out=rawT[:, sh, pt, :], in_=tp[:, :])
                        j = pt // 2
                        dst = a2a_ev_in if half == "ev" else a2a_od_in
                        nc.sync.dma_start(
                            out=dst[j].rearrange("(sh sp) q -> sp sh q", sh=2),
                            in_=rawT[:, :, pt, :])
                    # fire this half's AllToAll
                    if half == "ev":
                        nc.gpsimd.collective_compute(
                            "AllToAll", OP.bypass, replica_groups=[list(range(NC))],
                            ins=[a2a_ev_in[:, :, :]], outs=[a2a_ev_out[:, :, :]])
                        nc.sync.dma_start(
                            out=paT[:, :, 0:128],
                            in_=a2a_ev_out.rearrange("j (sh sp) q -> sp (j sh) q", sh=2))
                    else:
                        nc.gpsimd.collective_compute(
                            "AllToAll", OP.bypass, replica_groups=[list(range(NC))],
                            ins=[a2a_od_in[:, :, :]], outs=[a2a_od_out[:, :, :]])
                        nc.sync.dma_start(
                            out=paT[:, :, 128:256],
                            in_=a2a_od_out.rearrange("j (sh sp) q -> sp (j sh) q", sh=2))

                if debug:
                    nc.sync.dma_start(out=dbg["raw"].rearrange("p t s -> p (t s)"),
                                      in_=RAW.rearrange("p t s -> p (t s)"))

                # ---------------- phase 4: uc^T = [seq|1]^T @ pa ----------------
                with tc.tile_pool(name="ps_u", bufs=2, space="PSUM") as ps_u:
                    zp = ps_u.tile([1, PL], F32, space="PSUM", tag="zr")
                    for t in range(S // 128):
                        nc.tensor.matmul(
                            zp[:, :], seqx[:, t, H:H + 1], paT[:, t, :],
                            start=(t == 0), stop=(t == S // 128 - 1))
                    nc.vector.tensor_copy(out=zrow[:, :], in_=zp[:, :])
                    for mc in range(H // 128):
                        up = ps_u.tile([128, PL], F32, space="PSUM", tag="uc")
                        for t in range(S // 128):
                            nc.tensor.matmul(
                                up[:, :], seqx[:, t, mc * 128:(mc + 1) * 128],
                                paT[:, t, :], start=(t == 0), stop=(t == S // 128 - 1))
                        nc.vector.tensor_copy(out=ucb[:, mc, :], in_=up[:, :])
                if debug:
                    nc.sync.dma_start(
                        out=dbg["ctxuT"].rearrange("(t p) q -> p t q", p=128), in_=ucb)
                    nc.sync.dma_start(out=dbg["zrow"][:, :], in_=zrow)

                # recip(Z + 1e-6) -> broadcast to 128 partitions via K=1 matmul
                nc.vector.tensor_scalar_add(out=zrow[:, :], in0=zrow[:, :], scalar1=1e-6)
                nc.vector.reciprocal(out=zrow[:, :], in_=zrow[:, :])
                zrec_b = singles.tile([1, PL], BF16)
                nc.vector.tensor_copy(out=zrec_b, in_=zrow[:, :])
                with tc.tile_pool(name="ps_r", bufs=1, space="PSUM") as ps_r:
                    zrp = ps_r.tile([128, PL], F32, space="PSUM", tag="zrp")
                    nc.tensor.matmul(zrp[:, :], ones_row[:, :],
                                     zrec_b[:, :], start=True, stop=True)
                    nc.vector.tensor_copy(out=zrec, in_=zrp[:, :])
                for mc in range(H // 128):
                    nc.vector.tensor_tensor(out=ctxT[:, mc, :], in0=ucb[:, mc, :],
                                            in1=zrec[:, :], op=OP.mult)

                # ---------------- phase 5: z_s, z_o (ctx matmul shared) ----------------
                with tc.tile_pool(name="ps_z", bufs=2, space="PSUM") as ps_z, \
                     tc.tile_pool(name="zscr", bufs=2) as zscr:
                    for jt in range(PH // 128):
                        cps = ps_z.tile([128, PL], F32, space="PSUM", tag="cp")
                        for kt in range(H // 128):
                            nc.tensor.matmul(
                                cps[:, :], wcb[:, kt, jt * 128:(jt + 1) * 128],
                                ctxT[:, kt, :], start=(kt == 0), stop=(kt == H // 128 - 1))
                        cpsb = zscr.tile([128, PL], BF16, tag="cpsb")
                        nc.scalar.copy(out=cpsb[:, :], in_=cps[:, :])
                        for (ew, oh, bias, dst, tg) in ((EWh, ohh_t, bhs_t, zsT, "zs"),
                                                        (EWt, oht_t, bts_t, zoT, "zo")):
                            zps = ps_z.tile([128, PL], F32, space="PSUM", tag=tg)
                            for et in range(2):
                                nc.tensor.matmul(
                                    zps[:, :], ew[:, et, jt * 128:(jt + 1) * 128],
                                    oh[:, et, :], start=(et == 0), stop=(et == 1))
                            nc.vector.tensor_tensor(out=zps[:, :], in0=zps[:, :],
                                                    in1=cpsb[:, :], op=OP.add)
                            nc.scalar.activation(out=dst[:, jt, :], in_=zps[:, :],
                                                 func=AF.Tanh, bias=bias[:, jt:jt + 1])
                if debug:
                    nc.sync.dma_start(
                        out=dbg["zsT"].rearrange("(t p) q -> p t q", p=128), in_=zsT)
                # ---------------- phase 6: bilinear ----------------
                with tc.tile_pool(name="ps_b", bufs=3, space="PSUM") as ps_b, \
                     tc.tile_pool(name="bprod", bufs=2) as bprod:
                    lg = ps_b.tile([1, PL], F32, space="PSUM", tag="lg")
                    for jt in range(PH // 128):
                        ups = ps_b.tile([128, PL], F32, space="PSUM", tag="u")
                        for it in range(PH // 128):
                            nc.tensor.matmul(
                                ups[:, :], wbb[:, it, jt * 128:(jt + 1) * 128],
                                zsT[:, it, :], start=(it == 0), stop=(it == PH // 128 - 1))
                        pb = bprod.tile([128, PL], BF16, tag="pb")
                        nc.vector.tensor_tensor(out=pb[:, :], in0=ups[:, :],
                                                in1=zoT[:, jt, :], op=OP.mult)
                        nc.tensor.matmul(
                            lg[:, :], ones_col[:, :], pb[:, :],
                            start=(jt == 0), stop=(jt == PH // 128 - 1))
                    nc.vector.tensor_scalar_add(out=lg_sb[:, :], in0=lg[:, :],
                                                scalar1=bbil_t[:, 0:1])
                nc.sync.dma_start(out=out[:, :], in_=lg_sb)

    nc.finalize()
    return nc


def _get_nc(mt_ets, debug=False):
    key = ("nc", mt_ets, debug)
    if key not in _CACHE:
        _CACHE[key] = _build(mt_ets, debug)
    return _CACHE[key]


def _prep_in_maps(inputs):
    import ml_dtypes
    bf16 = ml_dtypes.bfloat16
    f8 = ml_dtypes.float8_e4m3

    att = np.asarray(inputs["attention"], np.float32)          # [16, 2048, 2048]
    seq = np.ascontiguousarray(np.asarray(inputs["sequence_output"], np.float32))
    mention_idx = np.asarray(inputs["mention_idx"], np.int32)  # [1024]
    entity_ids = np.asarray(inputs["entity_ids"], np.int32)    # [1024]
    pair_h = np.asarray(inputs["pair_h"], np.int32)            # [2048]
    pair_t = np.asarray(inputs["pair_t"], np.int32)

    counts = np.bincount(entity_ids, minlength=E).astype(np.float32)
    inv_cnt = 1.0 / np.maximum(counts, 1.0)
    invc = np.ascontiguousarray(inv_cnt.reshape(2, 128).T)     # [128, 2]

    ohe = np.zeros((NM, E), np.float32)
    ohe[np.arange(NM), entity_ids] = 1.0
    ohm = ohe                                                  # exact one-hot (fp8)
    has0r = (counts == 0).astype(np.float32)[None, :]

    # which entity-half chunks each mention tile touches (entity_ids sorted)
    eid_t = entity_ids.reshape(NMT, 128)
    mt_ets = tuple(
        tuple(sorted(set(int(e) // 128 for e in (eid_t[mt].min(), eid_t[mt].max()))
                     | set(range(int(eid_t[mt].min()) // 128,
                                 int(eid_t[mt].max()) // 128 + 1))))
        for mt in range(NMT))

    # pre-gathered mention rows: vals[p, mt*H:] = seq[mention_idx[mt*128+p]]
    vals_g = seq[mention_idx]                                   # [NM, H]
    vals_r = np.ascontiguousarray(
        vals_g.reshape(NMT, 128, H).transpose(1, 0, 2)).reshape(128, NMT * H).astype(bf16)

    # pre-gathered attention rows, s-sliced per core below
    attm = np.ascontiguousarray(
        att[:, mention_idx, :].transpose(1, 0, 2))              # [NM, HEADS, S]
    attm_bf = attm.astype(f8)

    order = np.argsort(pair_h, kind="stable")
    sph = pair_h[order]
    spt = pair_t[order]
    p_off = np.zeros((128, 2 * NPT), np.int32)
    for pt in range(NPT):
        seg = slice(pt * 128, (pt + 1) * 128)
        p_off[:, 2 * pt] = sph[seg]
        p_off[:, 2 * pt + 1] = spt[seg]

    shared = {
        "vals": vals_r,
        "seqb": seq.astype(bf16),
        "p_off": p_off,
        "ohe": ohe.astype(bf16),
        "ohm": ohm.astype(f8),
        "invc": invc,
        "has0r": has0r,
        "w_head": np.asarray(inputs["W_head"], np.float32).astype(bf16),
        "w_tail": np.asarray(inputs["W_tail"], np.float32).astype(bf16),
        "w_ctx": np.asarray(inputs["W_ctx"], np.float32).astype(bf16),
        "w_bil": np.asarray(inputs["W_bil"], np.float32).astype(bf16),
        "b_head": np.asarray(inputs["b_head"], np.float32).reshape(PH // 128, 128).T.copy(),
        "b_tail": np.asarray(inputs["b_tail"], np.float32).reshape(PH // 128, 128).T.copy(),
        "b_bil": np.asarray(inputs["b_bil"], np.float32).reshape(1, 1),
    }

    in_maps = []
    for k in range(NC):
        sk = k * SL
        # attg[p, mt, (h s)] = attm[mt*128+p, :, sk:sk+SL]
        attg_k = np.ascontiguousarray(
            attm_bf[:, :, sk:sk + SL].reshape(NMT, 128, HS).transpose(1, 0, 2)
        ).reshape(128, NMT * HS)
        ohh_kk = np.zeros((E, PL), np.float32)
        ohh_kk[sph[k * PL:(k + 1) * PL], np.arange(PL)] = 1.0
        oht_kk = np.zeros((E, PL), np.float32)
        oht_kk[spt[k * PL:(k + 1) * PL], np.arange(PL)] = 1.0
        m = dict(shared)
        m["attg"] = attg_k
        m["ohh_k"] = ohh_kk.astype(bf16)
        m["oht_k"] = oht_kk.astype(bf16)
        in_maps.append(m)
    return in_maps, mt_ets


def _run(inputs, trace=False, debug=False):
    _ensure_axon_profile_hook()
    from concourse.bass_utils import run_bass_kernel_spmd
    in_maps, mt_ets = _prep_in_maps(inputs)
    nc = _get_nc(mt_ets, debug)
    res = run_bass_kernel_spmd(nc, in_maps, list(range(NC)), trace=trace)
    sorted_logits = np.concatenate([np.asarray(res.results[k]["out"][0], np.float32)
                                    for k in range(NC)])
    order = np.argsort(np.asarray(inputs["pair_h"], np.int32), kind="stable")
    logits = np.empty(P, np.float32)
    logits[order] = sorted_logits
    return logits, res


def kernel(**inputs) -> np.ndarray:
    logits, _ = _run(inputs, trace=False)
    return logits


def kernel_traced(**inputs):
    logits, res = _run(inputs, trace=True)
    return logits, res


def kernel_debug(**inputs):
    logits, res = _run(inputs, trace=False, debug=True)
    return logits, res


# revision 26
# speedup vs baseline: 1.2034x; 1.0123x over previous
"""Trainium2 Bass kernel for nn_CandidateFilterModel (segment_reduce).

Strategy (8 cores, S-column sharding for the heavy phases, pair sharding for
the tail):
  - Core k owns sequence-column slice s_k = [256k, 256k+256).
  - Host pre-gathers attention/sequence mention rows (index prep only), so the
    device phase-1 loads are plain sequential HWDGE DMAs, not SWDGE gathers.
  - Phase 1: entity aggregation via one-hot matmuls (zero et-blocks skipped
    using the sorted entity_ids); ent_att -> DRAM table, ent_emb^T via
    exp/matmul/log (logsumexp).
  - Phase 2: pair products. Indirect-gather ent_att rows of head/tail entity
    per pair tile (SWDGE), multiply + tree-reduce over 16 heads -> RAW,
    PE-transpose to RAW^T.
  - Phase 3: TWO AllToAll chunks (even pair-tiles fire at ~50% of phase 2) so
    most of the collective hides under the gathers.
  - Phase 4: pairs-local tail with full weights (bf16 from host on HWDGE):
    contexts, z_s/z_o, bilinear.
All matmuls bf16 (fp32 PSUM accumulate).
"""
import sys
import types
import numpy as np

S, H, HEADS = 2048, 1024, 16
E, NM, P = 256, 1024, 2048
PH = 1024
NC = 8
SL = S // NC          # 256 s-columns per core
PL = P // NC          # 256 pairs per core
NMT = NM // 128       # 8 mention tiles
NPT = P // 128        # 16 pair tiles
HS = HEADS * SL       # 4096

_CACHE = {}


def _ensure_axon_profile_hook():
    """bass_utils' trace path imports antenv.axon_hooks, absent in this image."""
    if 'antenv.axon_hooks' in sys.modules:
        return
    try:
        import antenv.axon_hooks  # noqa: F401
        return
    except ImportError:
        pass
    mod = types.ModuleType('antenv.axon_hooks')
    holder = [None]
    mod.set_axon_ntff_profile_hook = lambda h: holder.__setitem__(0, h)
    mod.get_axon_ntff_profile_hook = lambda: holder[0]
    sys.modules['antenv.axon_hooks'] = mod
    try:
        from trn_agent_boot.trn_boot import _ntff_profile_via_ctypes
        hook = _ntff_profile_via_ctypes('/opt/axon/libaxon_pjrt.so')
        if hook is not None:
            mod.set_axon_ntff_profile_hook(hook)
    except Exception:
        pass


def _build(mt_ets, debug=False):
    """mt_ets: per mention-tile tuple of entity-half chunks (0/1) it touches;
    derived from the sorted entity_ids, identical across cores."""
    import concourse.bass as bass
    import concourse.bacc as bacc
    import concourse.tile as tile
    from concourse import mybir
    from concourse.masks import make_identity

    F32 = mybir.dt.float32
    BF16 = mybir.dt.bfloat16
    FP8 = mybir.dt.float8e4
    I32 = mybir.dt.int32
    AF = mybir.ActivationFunctionType
    OP = mybir.AluOpType
    DR = mybir.MatmulPerfMode.DoubleRow

    nc = bacc.Bacc(num_devices=NC)

    # ---------------- inputs ----------------
    attg = nc.declare_dram_parameter("attg", [128, NMT * HS], FP8, isOutput=False)
    vals = nc.declare_dram_parameter("vals", [128, NMT * H], BF16, isOutput=False)
    seqb = nc.declare_dram_parameter("seqb", [S, H], BF16, isOutput=False)
    p_off = nc.declare_dram_parameter("p_off", [128, 2 * NPT], I32, isOutput=False)
    ohe = nc.declare_dram_parameter("ohe", [NM, E], BF16, isOutput=False)
    ohm = nc.declare_dram_parameter("ohm", [NM, E], FP8, isOutput=False)
    invc = nc.declare_dram_parameter("invc", [128, 2], F32, isOutput=False)
    has0r = nc.declare_dram_parameter("has0r", [1, E], F32, isOutput=False)
    ohh_k = nc.declare_dram_parameter("ohh_k", [E, PL], BF16, isOutput=False)
    oht_k = nc.declare_dram_parameter("oht_k", [E, PL], BF16, isOutput=False)
    w_head = nc.declare_dram_parameter("w_head", [H, PH], BF16, isOutput=False)
    w_tail = nc.declare_dram_parameter("w_tail", [H, PH], BF16, isOutput=False)
    w_ctx = nc.declare_dram_parameter("w_ctx", [H, PH], BF16, isOutput=False)
    w_bil = nc.declare_dram_parameter("w_bil", [PH, PH], BF16, isOutput=False)
    b_head = nc.declare_dram_parameter("b_head", [128, PH // 128], F32, isOutput=False)
    b_tail = nc.declare_dram_parameter("b_tail", [128, PH // 128], F32, isOutput=False)
    b_bil = nc.declare_dram_parameter("b_bil", [1, 1], F32, isOutput=False)
    out = nc.declare_dram_parameter("out", [1, PL], F32, isOutput=True)

    dbg = {}
    if debug:
        dbg["ent_embT"] = nc.declare_dram_parameter("d_ent_embT", [H, E], BF16, isOutput=True)
        dbg["entA"] = nc.declare_dram_parameter("d_entA", [E, HS], FP8, isOutput=True)
        dbg["raw"] = nc.declare_dram_parameter("d_raw", [128, NPT, SL], BF16, isOutput=True)
        dbg["ctxuT"] = nc.declare_dram_parameter("d_ctxuT", [H, PL], BF16, isOutput=True)
        dbg["zrow"] = nc.declare_dram_parameter("d_zrow", [1, PL], F32, isOutput=True)
        dbg["zsT"] = nc.declare_dram_parameter("d_zsT", [PH, PL], BF16, isOutput=True)

    # internal DRAM
    entA_dram = nc.dram_tensor("entA_dram", [E, HS], FP8)
    a2a_ev_in = nc.dram_tensor("a2a_ev_in", [NC, SL, 128], BF16)
    a2a_ev_out = nc.dram_tensor("a2a_ev_out", [NC, SL, 128], BF16)
    a2a_od_in = nc.dram_tensor("a2a_od_in", [NC, SL, 128], BF16)
    a2a_od_out = nc.dram_tensor("a2a_od_out", [NC, SL, 128], BF16)

    with tile.TileContext(nc) as tc:
        with tc.tile_pool(name="singles", bufs=1) as singles:
            entTe = singles.tile([128, H // 128, E], BF16)  # ent_emb^T [hcol-part, hc, e]
            RAW = singles.tile([128, NPT, SL], BF16)        # [p-row, pt, s]
            rawT = singles.tile([128, 2, NPT, 128], BF16)   # [s-part, sh, pt, p-row]
            paT = singles.tile([128, S // 128, PL], BF16)   # RAW^T for my pairs, all s
            ucb = singles.tile([128, H // 128, PL], BF16)   # contexts^T (unnormalized)
            ctxT = singles.tile([128, H // 128, PL], BF16)
            zsT = singles.tile([128, PH // 128, PL], BF16)
            zoT = singles.tile([128, PH // 128, PL], BF16)
            EWh = singles.tile([128, 2, PH], BF16)          # ent_emb @ W_head [e-part, et, PH]
            EWt = singles.tile([128, 2, PH], BF16)
            zrow = singles.tile([1, PL], F32)
            zrec = singles.tile([128, PL], BF16)
            lg_sb = singles.tile([1, PL], F32)

            # gather/product pools open before (and stay below) the phase-1
            # pools: their SBUF never aliases phase-1 tiles, so the SWDGE
            # gathers never pick up false WAR waits on agg/lse/EW readers.
            from contextlib import ExitStack
            gstack = ExitStack()
            pg = gstack.enter_context(tc.tile_pool(name="pgP", bufs=2))
            prod = gstack.enter_context(tc.tile_pool(name="prodP", bufs=1))

            # ---------------- phase 1: entity aggregation ----------------
            with tc.tile_pool(name="early", bufs=1) as early, \
                 tc.tile_pool(name="ps_a", bufs=1, space="PSUM") as ps_a, \
                 tc.tile_pool(name="wpoolA", bufs=1) as wpoolA:
                # one-hots first (small, needed by the first agg matmul), then
                # the big attg chunks; everything else rides the scalar queue
                ohm_t = early.tile([128, NMT, E], FP8)
                nc.sync.dma_start(out=ohm_t, in_=ohm.rearrange("(t p) e -> p t e", p=128))
                ohe_t = early.tile([128, NMT, E], BF16)
                nc.sync.dma_start(out=ohe_t, in_=ohe.rearrange("(t p) e -> p t e", p=128))
                invc_t = singles.tile([128, 2], F32)
                nc.sync.dma_start(out=invc_t, in_=invc[:, :])
                attg_t = early.tile([128, NMT, HS], FP8)
                attg_v = attg.rearrange("p (t w) -> p t w", t=NMT)
                for hg in range(2):
                    nc.sync.dma_start(out=attg_t[:, :, hg * 2048:(hg + 1) * 2048],
                                      in_=attg_v[:, :, hg * 2048:(hg + 1) * 2048])
                vals_t = early.tile([128, NMT, H], BF16)
                nc.scalar.dma_start(out=vals_t, in_=vals.rearrange("p (t h) -> p t h", t=NMT))

                # ---------------- phase 0: small loads (scalar queue) -------
                p_off_t = singles.tile([128, 2 * NPT], I32)
                nc.scalar.dma_start(out=p_off_t, in_=p_off[:, :])
                ohh_t = singles.tile([128, 2, PL], BF16)
                nc.scalar.dma_start(out=ohh_t, in_=ohh_k.rearrange("(t p) q -> p t q", p=128))
                oht_t = singles.tile([128, 2, PL], BF16)
                nc.scalar.dma_start(out=oht_t, in_=oht_k.rearrange("(t p) q -> p t q", p=128))
                has0b = singles.tile([128, E], F32)
                nc.scalar.dma_start(out=has0b, in_=has0r[:, :].to_broadcast([128, E]))
                bhs_t = singles.tile([128, PH // 128], F32)
                nc.scalar.dma_start(out=bhs_t, in_=b_head[:, :])
                bts_t = singles.tile([128, PH // 128], F32)
                nc.scalar.dma_start(out=bts_t, in_=b_tail[:, :])
                bbil_t = singles.tile([1, 1], F32)
                nc.scalar.dma_start(out=bbil_t, in_=b_bil[:, :])
                whb = wpoolA.tile([128, H // 128, PH], BF16)
                nc.scalar.dma_start(out=whb, in_=w_head.rearrange("(t p) n -> p t n", p=128))
                wtb = wpoolA.tile([128, H // 128, PH], BF16)
                nc.scalar.dma_start(out=wtb, in_=w_tail.rearrange("(t p) n -> p t n", p=128))
                ident = singles.tile([128, 128], BF16)
                make_identity(nc, ident[:, :])
                warm = singles.tile([1, 8], F32)
                nc.vector.memset(warm[:, :], 1.0)
                nc.scalar.activation(out=warm[:, :], in_=warm[:, :], func=AF.Exp)
                nc.scalar.activation(out=warm[:, :], in_=warm[:, :], func=AF.Ln)
                nc.scalar.activation(out=warm[:, :], in_=warm[:, :], func=AF.Tanh)
                ones_col = singles.tile([128, 1], BF16)
                nc.vector.memset(ones_col[:, :], 1.0)
                ones_row = singles.tile([1, 128], BF16)
                nc.vector.memset(ones_row[:, :], 1.0)

                entA_sb = early.tile([128, 2, HS], FP8)   # [e-part, et, (h s)]
                ev = vals_t                               # exp applied in place

                # agg matmuls (fp8 DoubleRow over mention-tile pairs), skipping
                # (mt, et) blocks that the sorted entity_ids can never touch
                for hg in range(2):
                    for et in range(2):
                        mts = [mt for mt in range(NMT) if et in mt_ets[mt]]
                        groups = []
                        i = 0
                        while i < len(mts):
                            if i + 1 < len(mts) and mts[i + 1] == mts[i] + 1:
                                groups.append((mts[i], 2))
                                i += 2
                            else:
                                groups.append((mts[i], 1))
                                i += 1
                        if not groups:
                            nc.vector.memset(
                                entA_sb[:, et, hg * 2048:(hg + 1) * 2048], 0.0)
                        else:
                            pa = ps_a.tile([128, 8 * SL], F32, space="PSUM", tag="agg")
                            for i, (mt, w) in enumerate(groups):
                                for nch in range(4):  # 2048 = 4 x 512
                                    cs = hg * 2048 + nch * 512
                                    if w == 2:
                                        nc.tensor.matmul(
                                            pa[:, nch * 512:(nch + 1) * 512],
                                            ohm_t[:, mt:mt + 2, et * 128:(et + 1) * 128],
                                            attg_t[:, mt:mt + 2, cs:cs + 512],
                                            start=(i == 0), stop=(i == len(groups) - 1),
                                            perf_mode=DR)
                                    else:
                                        nc.tensor.matmul(
                                            pa[:, nch * 512:(nch + 1) * 512],
                                            ohm_t[:, mt, et * 128:(et + 1) * 128],
                                            attg_t[:, mt, cs:cs + 512],
                                            start=(i == 0), stop=(i == len(groups) - 1))
                            nc.vector.tensor_scalar_mul(
                                out=entA_sb[:, et, hg * 2048:(hg + 1) * 2048],
                                in0=pa[:, :], scalar1=invc_t[:, et:et + 1])
                        nc.sync.dma_start(
                            out=entA_dram.rearrange("(t p) w -> p t w", p=128)[
                                :, et, hg * 2048:(hg + 1) * 2048],
                            in_=entA_sb[:, et, hg * 2048:(hg + 1) * 2048])
                if debug:
                    nc.sync.dma_start(
                        out=dbg["entA"].rearrange("(t p) w -> p t w", p=128), in_=entA_sb)

                # ---------------- phase 1b: logsumexp ----------------
                for mt in range(NMT):
                    nc.scalar.activation(out=ev[:, mt, :], in_=vals_t[:, mt, :],
                                         func=AF.Exp)
                with tc.tile_pool(name="ps_s", bufs=2, space="PSUM") as ps_s:
                    for hc in range(H // 128):
                        sp = ps_s.tile([128, E], F32, space="PSUM", tag="sums")
                        for mt in range(NMT):
                            nc.tensor.matmul(
                                sp[:, :], ev[:, mt, hc * 128:(hc + 1) * 128],
                                ohe_t[:, mt, :], start=(mt == 0), stop=(mt == NMT - 1))
                        nc.vector.tensor_tensor(out=sp[:, :], in0=sp[:, :],
                                                in1=has0b[:, :], op=OP.add)
                        nc.scalar.activation(out=entTe[:, hc, :], in_=sp[:, :], func=AF.Ln)
                if debug:
                    nc.sync.dma_start(
                        out=dbg["ent_embT"].rearrange("(t p) e -> p t e", p=128), in_=entTe)

                # ---------------- phase 4b: EW = ent_emb @ W (fills gather window) ----
                with tc.tile_pool(name="ps_e", bufs=2, space="PSUM") as ps_e:
                    for (wsb, dst) in ((whb, EWh), (wtb, EWt)):
                        for et in range(2):
                            ep = ps_e.tile([128, PH], F32, space="PSUM", tag="ew")
                            for kt in range(H // 128):
                                for nch in range(2):
                                    nc.tensor.matmul(
                                        ep[:, nch * 512:(nch + 1) * 512],
                                        entTe[:, kt, et * 128:(et + 1) * 128],
                                        wsb[:, kt, nch * 512:(nch + 1) * 512],
                                        start=(kt == 0), stop=(kt == H // 128 - 1))
                            nc.scalar.copy(out=dst[:, et, :], in_=ep[:, :])

            # ---------------- phase 2: pair products ----------------
            # pt order: evens then odds; the even-half AllToAll fires at ~50%.
            with tc.tile_pool(name="wpoolB", bufs=1) as wpoolB:
                wcb = wpoolB.tile([128, H // 128, PH], BF16)
                nc.scalar.dma_start(out=wcb, in_=w_ctx.rearrange("(t p) n -> p t n", p=128))
                seqx = wpoolB.tile([128, S // 128, H + 1], BF16)
                nc.scalar.dma_start(
                    out=seqx[:, :, 0:H], in_=seqb.rearrange("(t p) h -> p t h", p=128))
                nc.vector.memset(seqx[:, :, H:H + 1], 1.0)
                wbb = wpoolB.tile([128, PH // 128, PH], BF16)
                nc.scalar.dma_start(out=wbb, in_=w_bil.rearrange("(t p) n -> p t n", p=128))

                with tc.tile_pool(name="ps_t", bufs=4, space="PSUM") as ps_t:
                  for half, pts in (("ev", range(0, NPT, 2)), ("od", range(1, NPT, 2))):
                    for pt in pts:
                        th = pg.tile([128, HS], BF16, tag="th")
                        nc.gpsimd.indirect_dma_start(
                            out=th[:, :], out_offset=None, in_=entA_dram[:, :],
                            in_offset=bass.IndirectOffsetOnAxis(
                                ap=p_off_t[:, 2 * pt:2 * pt + 1], axis=0))
                        tt = pg.tile([128, HS], BF16, tag="tt")
                        nc.gpsimd.indirect_dma_start(
                            out=tt[:, :], out_offset=None, in_=entA_dram[:, :],
                            in_offset=bass.IndirectOffsetOnAxis(
                                ap=p_off_t[:, 2 * pt + 1:2 * pt + 2], axis=0))
                        pr = prod.tile([128, HS], BF16, tag="pr")
                        t1 = prod.tile([128, 8 * SL], BF16, tag="t1")
                        t2 = prod.tile([128, 4 * SL], BF16, tag="t2")
                        t3 = prod.tile([128, 2 * SL], BF16, tag="t3")
                        nc.vector.tensor_tensor(out=pr[:, :], in0=th[:, :], in1=tt[:, :],
                                                op=OP.mult)
                        nc.vector.tensor_tensor(out=t1[:, :], in0=pr[:, :8 * SL],
                                                in1=pr[:, 8 * SL:], op=OP.add)
                        nc.vector.tensor_tensor(out=t2[:, :], in0=t1[:, :4 * SL],
                                                in1=t1[:, 4 * SL:], op=OP.add)
                        nc.vector.tensor_tensor(out=t3[:, :], in0=t2[:, :2 * SL],
                                                in1=t2[:, 2 * SL:], op=OP.add)
                        nc.vector.tensor_tensor(out=RAW[:, pt, :], in0=t3[:, :SL],
                                                in1=t3[:, SL:], op=OP.add)
                        # transpose inline (PE is idle during products)
                        for sh in range(2):
                            tp = ps_t.tile([128, 128], BF16, space="PSUM", tag="tp")
                            nc.tensor.transpose(
                                out=tp[:, :], in_=RAW[:, pt, sh * 128:(sh + 1) * 128],
                                identity=ident[:, :])
                            nc.scalar.copy(out=rawT[:, sh, pt, :], in_=tp[:, :])
                        j = pt // 2
                        dst = a2a_ev_in if half == "ev" else a2a_od_in
                        nc.sync.dma_start(
                            out=dst[j].rearrange("(sh sp) q -> sp sh q", sh=2),
                            in_=rawT[:, :, pt, :])
                    # fire this half's AllToAll
                    if half == "ev":
                        nc.gpsimd.collective_compute(
                            "AllToAll", OP.bypass, replica_groups=[list(range(NC))],
                            ins=[a2a_ev_in[:, :, :]], outs=[a2a_ev_out[:, :, :]])
                        nc.sync.dma_start(
                            out=paT[:, :, 0:128],
                            in_=a2a_ev_out.rearrange("j (sh sp) q -> sp (j sh) q", sh=2))
                    else:
                        nc.gpsimd.collective_compute(
                            "AllToAll", OP.bypass, replica_groups=[list(range(NC))],
                            ins=[a2a_od_in[:, :, :]], outs=[a2a_od_out[:, :, :]])
                        nc.sync.dma_start(
                            out=paT[:, :, 128:256],
                            in_=a2a_od_out.rearrange("j (sh sp) q -> sp (j sh) q", sh=2))

                if debug:
                    nc.sync.dma_start(out=dbg["raw"].rearrange("p t s -> p (t s)"),
                                      in_=RAW.rearrange("p t s -> p (t s)"))

                # ---------------- phase 4: uc^T = [seq|1]^T @ pa ----------------
                with tc.tile_pool(name="ps_u", bufs=2, space="PSUM") as ps_u:
                    zp = ps_u.tile([1, PL], F32, space="PSUM", tag="zr")
                    for t in range(S // 128):
                        nc.tensor.matmul(
                            zp[:, :], seqx[:, t, H:H + 1], paT[:, t, :],
                            start=(t == 0), stop=(t == S // 128 - 1))
                    nc.vector.tensor_copy(out=zrow[:, :], in_=zp[:, :])
                    for mc in range(H // 128):
                        up = ps_u.tile([128, PL], F32, space="PSUM", tag="uc")
                        for t in range(S // 128):
                            nc.tensor.matmul(
                                up[:, :], seqx[:, t, mc * 128:(mc + 1) * 128],
                                paT[:, t, :], start=(t == 0), stop=(t == S // 128 - 1))
                        nc.vector.tensor_copy(out=ucb[:, mc, :], in_=up[:, :])
                if debug:
                    nc.sync.dma_start(
                        out=dbg["ctxuT"].rearrange("(t p) q -> p t q", p=128), in_=ucb)
                    nc.sync.dma_start(out=dbg["zrow"][:, :], in_=zrow)

                # recip(Z + 1e-6) -> broadcast to 128 partitions via K=1 matmul
                nc.vector.tensor_scalar_add(out=zrow[:, :], in0=zrow[:, :], scalar1=1e-6)
                nc.vector.reciprocal(out=zrow[:, :], in_=zrow[:, :])
                zrec_b = singles.tile([1, PL], BF16)
                nc.vector.tensor_copy(out=zrec_b, in_=zrow[:, :])
                with tc.tile_pool(name="ps_r", bufs=1, space="PSUM") as ps_r:
                    zrp = ps_r.tile([128, PL], F32, space="PSUM", tag="zrp")
                    nc.tensor.matmul(zrp[:, :], ones_row[:, :],
                                     zrec_b[:, :], start=True, stop=True)
                    nc.vector.tensor_copy(out=zrec, in_=zrp[:, :])
                for mc in range(H // 128):
                    nc.vector.tensor_tensor(out=ctxT[:, mc, :], in0=ucb[:, mc, :],
                                            in1=zrec[:, :], op=OP.mult)

                # ---------------- phase 5: z_s, z_o (ctx matmul shared) ----------------
                with tc.tile_pool(name="ps_z", bufs=2, space="PSUM") as ps_z, \
                     tc.tile_pool(name="zscr", bufs=2) as zscr:
                    for jt in range(PH // 128):
                        cps = ps_z.tile([128, PL], F32, space="PSUM", tag="cp")
                        for kt in range(H // 128):
                            nc.tensor.matmul(
                                cps[:, :], wcb[:, kt, jt * 128:(jt + 1) * 128],
                                ctxT[:, kt, :], start=(kt == 0), stop=(kt == H // 128 - 1))
                        cpsb = zscr.tile([128, PL], BF16, tag="cpsb")
                        nc.scalar.copy(out=cpsb[:, :], in_=cps[:, :])
                        for (ew, oh, bias, dst, tg) in ((EWh, ohh_t, bhs_t, zsT, "zs"),
                                                        (EWt, oht_t, bts_t, zoT, "zo")):
                            zps = ps_z.tile([128, PL], F32, space="PSUM", tag=tg)
                            for et in range(2):
                                nc.tensor.matmul(
                                    zps[:, :], ew[:, et, jt * 128:(jt + 1) * 128],
                                    oh[:, et, :], start=(et == 0), stop=(et == 1))
                            nc.vector.tensor_tensor(out=zps[:, :], in0=zps[:, :],
                                                    in1=cpsb[:, :], op=OP.add)
                            nc.scalar.activation(out=dst[:, jt, :], in_=zps[:, :],
                                                 func=AF.Tanh, bias=bias[:, jt:jt + 1])
                if debug:
                    nc.sync.dma_start(
                        out=dbg["zsT"].rearrange("(t p) q -> p t q", p=128), in_=zsT)
                # ---------------- phase 6: bilinear ----------------
                with tc.tile_pool(name="ps_b", bufs=3, space="PSUM") as ps_b, \
                     tc.tile_pool(name="bprod", bufs=2) as bprod:
                    lg = ps_b.tile([1, PL], F32, space="PSUM", tag="lg")
                    for jt in range(PH // 128):
                        ups = ps_b.tile([128, PL], F32, space="PSUM", tag="u")
                        for it in range(PH // 128):
                            nc.tensor.matmul(
                                ups[:, :], wbb[:, it, jt * 128:(jt + 1) * 128],
                                zsT[:, it, :], start=(it == 0), stop=(it == PH // 128 - 1))
                        pb = bprod.tile([128, PL], BF16, tag="pb")
                        nc.vector.tensor_tensor(out=pb[:, :], in0=ups[:, :],
                                                in1=zoT[:, jt, :], op=OP.mult)
                        nc.tensor.matmul(
                            lg[:, :], ones_col[:, :], pb[:, :],
                            start=(jt == 0), stop=(jt == PH // 128 - 1))
                    nc.vector.tensor_scalar_add(out=lg_sb[:, :], in0=lg[:, :],
                                                scalar1=bbil_t[:, 0:1])
                nc.sync.dma_start(out=out[:, :], in_=lg_sb)
            gstack.close()

    nc.finalize()
    return nc


def _get_nc(mt_ets, debug=False):
    key = ("nc", mt_ets, debug)
    if key not in _CACHE:
        _CACHE[key] = _build(mt_ets, debug)
    return _CACHE[key]


def _prep_in_maps(inputs):
    import ml_dtypes
    bf16 = ml_dtypes.bfloat16
    f8 = ml_dtypes.float8_e4m3

    att = np.asarray(inputs["attention"], np.float32)          # [16, 2048, 2048]
    seq = np.ascontiguousarray(np.asarray(inputs["sequence_output"], np.float32))
    mention_idx = np.asarray(inputs["mention_idx"], np.int32)  # [1024]
    entity_ids = np.asarray(inputs["entity_ids"], np.int32)    # [1024]
    pair_h = np.asarray(inputs["pair_h"], np.int32)            # [2048]
    pair_t = np.asarray(inputs["pair_t"], np.int32)

    counts = np.bincount(entity_ids, minlength=E).astype(np.float32)
    inv_cnt = 1.0 / np.maximum(counts, 1.0)
    invc = np.ascontiguousarray(inv_cnt.reshape(2, 128).T)     # [128, 2]

    ohe = np.zeros((NM, E), np.float32)
    ohe[np.arange(NM), entity_ids] = 1.0
    ohm = ohe                                                  # exact one-hot (fp8)
    has0r = (counts == 0).astype(np.float32)[None, :]

    # which entity-half chunks each mention tile touches (entity_ids sorted)
    eid_t = entity_ids.reshape(NMT, 128)
    mt_ets = tuple(
        tuple(sorted(set(int(e) // 128 for e in (eid_t[mt].min(), eid_t[mt].max()))
                     | set(range(int(eid_t[mt].min()) // 128,
                                 int(eid_t[mt].max()) // 128 + 1))))
        for mt in range(NMT))

    # pre-gathered mention rows: vals[p, mt*H:] = seq[mention_idx[mt*128+p]]
    vals_g = seq[mention_idx]                                   # [NM, H]
    vals_r = np.ascontiguousarray(
        vals_g.reshape(NMT, 128, H).transpose(1, 0, 2)).reshape(128, NMT * H).astype(bf16)

    # pre-gathered attention rows, s-sliced per core below
    attm = np.ascontiguousarray(
        att[:, mention_idx, :].transpose(1, 0, 2))              # [NM, HEADS, S]
    attm_bf = attm.astype(f8)

    order = np.argsort(pair_h, kind="stable")
    sph = pair_h[order]
    spt = pair_t[order]
    p_off = np.zeros((128, 2 * NPT), np.int32)
    for pt in range(NPT):
        seg = slice(pt * 128, (pt + 1) * 128)
        p_off[:, 2 * pt] = sph[seg]
        p_off[:, 2 * pt + 1] = spt[seg]

    shared = {
        "vals": vals_r,
        "seqb": seq.astype(bf16),
        "p_off": p_off,
        "ohe": ohe.astype(bf16),
        "ohm": ohm.astype(f8),
        "invc": invc,
        "has0r": has0r,
        "w_head": np.asarray(inputs["W_head"], np.float32).astype(bf16),
        "w_tail": np.asarray(inputs["W_tail"], np.float32).astype(bf16),
        "w_ctx": np.asarray(inputs["W_ctx"], np.float32).astype(bf16),
        "w_bil": np.asarray(inputs["W_bil"], np.float32).astype(bf16),
        "b_head": np.asarray(inputs["b_head"], np.float32).reshape(PH // 128, 128).T.copy(),
        "b_tail": np.asarray(inputs["b_tail"], np.float32).reshape(PH // 128, 128).T.copy(),
        "b_bil": np.asarray(inputs["b_bil"], np.float32).reshape(1, 1),
    }

    in_maps = []
    for k in range(NC):
        sk = k * SL
        # attg[p, mt, (h s)] = attm[mt*128+p, :, sk:sk+SL]
        attg_k = np.ascontiguousarray(
            attm_bf[:, :, sk:sk + SL].reshape(NMT, 128, HS).transpose(1, 0, 2)
        ).reshape(128, NMT * HS)
        ohh_kk = np.zeros((E, PL), np.float32)
        ohh_kk[sph[k * PL:(k + 1) * PL], np.arange(PL)] = 1.0
        oht_kk = np.zeros((E, PL), np.float32)
        oht_kk[spt[k * PL:(k + 1) * PL], np.arange(PL)] = 1.0
        m = dict(shared)
        m["attg"] = attg_k
        m["ohh_k"] = ohh_kk.astype(bf16)
        m["oht_k"] = oht_kk.astype(bf16)
        in_maps.append(m)
    return in_maps, mt_ets


def _run(inputs, trace=False, debug=False):
    _ensure_axon_profile_hook()
    from concourse.bass_utils import run_bass_kernel_spmd
    in_maps, mt_ets = _prep_in_maps(inputs)
    nc = _get_nc(mt_ets, debug)
    res = run_bass_kernel_spmd(nc, in_maps, list(range(NC)), trace=trace)
    sorted_logits = np.concatenate([np.asarray(res.results[k]["out"][0], np.float32)
                                    for k in range(NC)])
    order = np.argsort(np.asarray(inputs["pair_h"], np.int32), kind="stable")
    logits = np.empty(P, np.float32)
    logits[order] = sorted_logits
    return logits, res


def kernel(**inputs) -> np.ndarray:
    logits, _ = _run(inputs, trace=False)
    return logits


def kernel_traced(**inputs):
    logits, res = _run(inputs, trace=True)
    return logits, res


def kernel_debug(**inputs):
    logits, res = _run(inputs, trace=False, debug=True)
    return logits, res
